# revision 4
# baseline (speedup 1.0000x reference)
"""Trainium2 Bass kernel for GroupAttention.

Reference computation (B=4, N=8192, C=1024, H=16 heads, Dh=64, groups of
g=4 consecutive tokens):
    qkv = x @ w_qkv                      # [B,N,3C]
    per (batch, group, head): S = (q*Dh^-0.5) @ k.T   (4x4)
    P = softmax(S, axis=-1); o = P @ v
    y = o @ w_proj + b_proj

Strategy: data-parallel over the 32768 tokens -> 4096 tokens/core on 8
NeuronCores (group locality preserved: 4096 % (8192) token chunks never
split a 4-token group or batch row).

Per core, per 512-token window (= 128 groups):
  - DMA x window (bf16, host-cast), PE-transpose to feature-major Xt.
  - qkv matmul with the *stationary* operand Xt[:, n::4] (tokens at
    position n within their group, strided) so PSUM comes out
    group-major: [128 groups, outc]. Copy+cast to bf16 SBUF tiles
    Q/K/V laid out [group, (pos, head, dh)].
  - Attention entirely on vector engine per (key position m): mult +
    segmented reduce over dh -> scores; exp on scalar engine; sum/recip/
    normalize; AV as broadcast mult + accumulate.
  - PE-transpose O back to feature-major, proj matmul (+bias via a K=1
    matmul with a ones row), copy PSUM->SBUF, DMA out (fp32).

The 1/sqrt(Dh) scale is folded into the q-columns of w_qkv on the host.
Matmul/attention inputs are bf16 (cast host-side); accumulations are
fp32 (PSUM / DVE internal).
"""

import numpy as np
import ml_dtypes

import concourse.bass as bass
import concourse.bacc as bacc
import concourse.mybir as mybir
import concourse.tile as tile
from concourse.bass_utils import run_bass_kernel_spmd

BF16 = mybir.dt.bfloat16
F32 = mybir.dt.float32
AF = mybir.ActivationFunctionType
ALU = mybir.AluOpType
AX = mybir.AxisListType

B, N, C = 4, 8192, 1024
H, DH, GSZ = 16, 64, 4
NCORES = 8
T_CORE = (B * N) // NCORES  # 4096 tokens per core
WIN = 512                   # tokens per window (= 128 groups)
G128 = WIN // GSZ           # 128 groups per window
KT = C // 128               # 8 contraction tiles of 128
OUT3 = 3 * C                # 3072
NCH = OUT3 // 512           # 6 qkv output chunks of 512


def group_attn_kernel(tc, y, x, wqkv, wproj, bias, ident, ones, t_core=T_CORE):
    """Emit the per-core kernel. All args are DRAM APs:
    y [t_core, C] f32 out; x [t_core, C] bf16; wqkv [C, 3C] bf16 (q cols
    pre-scaled); wproj [C, C] bf16; bias [1, C] bf16; ident [128,128]
    bf16; ones [1,128] bf16.
    """
    nc = tc.nc
    nwin = t_core // WIN

    from contextlib import ExitStack

    with ExitStack() as ctx:
        ep = ctx.enter_context

        const = ep(tc.tile_pool(name="const", bufs=1))
        xpool = ep(tc.tile_pool(name="x", bufs=2))
        xtpool = ep(tc.tile_pool(name="xt", bufs=2))
        qpool = ep(tc.tile_pool(name="qb", bufs=1))
        kpool = ep(tc.tile_pool(name="kb", bufs=1))
        vpool = ep(tc.tile_pool(name="vb", bufs=1))
        spool = ep(tc.tile_pool(name="soft", bufs=2))
        prodpool = ep(tc.tile_pool(name="prod", bufs=2))
        opool = ep(tc.tile_pool(name="o", bufs=2))
        otpool = ep(tc.tile_pool(name="ot", bufs=2))
        ypool = ep(tc.tile_pool(name="y", bufs=4))

        ps_qkv = ep(tc.tile_pool(name="ps_qkv", bufs=3, space="PSUM"))
        ps_t = ep(tc.tile_pool(name="ps_t", bufs=2, space="PSUM"))
        ps_y = ep(tc.tile_pool(name="ps_y", bufs=2, space="PSUM"))

        # ---- constants: weights, bias, identity ----
        wqkv_sb = const.tile([128, KT * OUT3], BF16)   # 48KB/part
        nc.sync.dma_start(
            wqkv_sb[:].rearrange("p (k c) -> p k c", k=KT),
            wqkv.rearrange("(k p) c -> p k c", p=128),
        )
        wproj_sb = const.tile([128, KT * C], BF16)     # 16KB/part
        nc.sync.dma_start(
            wproj_sb[:].rearrange("p (k c) -> p k c", k=KT),
            wproj.rearrange("(k p) c -> p k c", p=128),
        )
        bias_sb = const.tile([1, C], BF16)
        nc.sync.dma_start(bias_sb[:], bias[:])
        ident_sb = const.tile([128, 128], BF16)
        nc.sync.dma_start(ident_sb[:], ident[:])
        ones_sb = const.tile([1, 128], BF16)
        nc.sync.dma_start(ones_sb[:], ones[:])

        for w in range(nwin):
            # ---- load X window [512, C] -> [128, (t, c)] ----
            x_t = xpool.tile([128, 4 * C], BF16)
            nc.sync.dma_start(
                x_t[:].rearrange("p (t c) -> p t c", t=4),
                x[w * WIN:(w + 1) * WIN, :].rearrange("(t p) c -> p t c", p=128),
            )

            # ---- transpose to feature-major Xt: KT tiles [128c, 512 tok] ----
            xt = xtpool.tile([128, KT * WIN], BF16)
            for k in range(KT):
                pst = ps_t.tile([128, WIN], BF16)
                for t in range(4):
                    nc.tensor.transpose(
                        pst[:, t * 128:(t + 1) * 128],
                        x_t[:, t * C + k * 128: t * C + (k + 1) * 128],
                        ident_sb[:],
                    )
                nc.scalar.copy(xt[:, k * WIN:(k + 1) * WIN], pst[:])

            # ---- qkv matmuls, group-major output ----
            qb = qpool.tile([128, 4 * C], BF16)   # [g, (n, h, dh)]
            kb = kpool.tile([128, 4 * C], BF16)   # [g, (m, h, dh)]
            vb = vpool.tile([128, 4 * C], BF16)   # [g, (m, h, dh)]
            dest_of = {0: qb, 1: kb, 2: vb}
            for n in range(GSZ):
                for ch in range(NCH):
                    ps = ps_qkv.tile([128, 512], F32)
                    for k in range(KT):
                        nc.tensor.matmul(
                            ps[:],
                            lhsT=xt[:, k * WIN + n: k * WIN + WIN: GSZ],
                            rhs=wqkv_sb[:, k * OUT3 + ch * 512: k * OUT3 + (ch + 1) * 512],
                            start=(k == 0),
                            stop=(k == KT - 1),
                        )
                    which, hblk = divmod(ch, 2)
                    dst = dest_of[which][:, n * C + hblk * 512: n * C + (hblk + 1) * 512]
                    if which == 2:
                        nc.vector.tensor_copy(dst, ps[:])
                    else:
                        nc.scalar.copy(dst, ps[:])

            # ---- attention (per window, all 16 heads) ----
            # scores: S[g, (m, n, h)] = sum_dh Q[g,n,h,:] * K[g,m,h,:]
            s_f = spool.tile([128, 256], F32, tag="s")
            q_v = qb[:].rearrange("p (n h d) -> p n h d", n=GSZ, h=H)
            for m in range(GSZ):
                prod = prodpool.tile([128, 4 * C], BF16)
                k_v = (
                    kb[:, m * C:(m + 1) * C]
                    .rearrange("p (h d) -> p h d", h=H)
                    .unsqueeze(1)
                    .broadcast_to([128, GSZ, H, DH])
                )
                prod_v = prod[:].rearrange("p (n h d) -> p n h d", n=GSZ, h=H)
                nc.vector.tensor_mul(prod_v, q_v, k_v)
                nc.vector.tensor_reduce(
                    s_f[:, m * 64:(m + 1) * 64].rearrange("p (n h) -> p n h", n=GSZ),
                    prod_v,
                    axis=AX.X,
                    op=ALU.add,
                )
            # softmax over m (no max-subtraction: |S| is O(5) here)
            e_f = spool.tile([128, 256], F32, tag="e")
            nc.scalar.activation(e_f[:], s_f[:], AF.Exp)
            z_f = spool.tile([128, 64], F32, tag="z")
            e_nhm = e_f[:].rearrange("p (m n h) -> p n h m", m=GSZ, n=GSZ)
            nc.vector.tensor_reduce(
                z_f[:].rearrange("p (n h) -> p n h", n=GSZ), e_nhm,
                axis=AX.X, op=ALU.add,
            )
            r_f = spool.tile([128, 64], F32, tag="r")
            nc.vector.reciprocal(r_f[:], z_f[:])
            pb = spool.tile([128, 256], BF16, tag="pb")  # [g, (n, h, m)]
            r_v = (
                r_f[:].rearrange("p (n h) -> p n h", n=GSZ)
                .unsqueeze(3)
                .broadcast_to([128, GSZ, H, GSZ])
            )
            pb_v = pb[:].rearrange("p (n h m) -> p n h m", n=GSZ, h=H)
            nc.vector.tensor_mul(pb_v, e_nhm, r_v)

            # AV: O[g, (n, h, d)] = sum_m P[g,n,h,m] * V[g,m,h,:]
            ob = opool.tile([128, 4 * C], BF16)
            ob_v = ob[:].rearrange("p (n h d) -> p n h d", n=GSZ, h=H)
            for m in range(GSZ):
                v_v = (
                    vb[:, m * C:(m + 1) * C]
                    .rearrange("p (h d) -> p h d", h=H)
                    .unsqueeze(1)
                    .broadcast_to([128, GSZ, H, DH])
                )
                p_v = (
                    pb[:, m: 256: GSZ]
                    .rearrange("p (n h) -> p n h", n=GSZ)
                    .unsqueeze(3)
                    .broadcast_to([128, GSZ, H, DH])
                )
                if m == 0:
                    nc.vector.tensor_mul(ob_v, v_v, p_v)
                else:
                    prod2 = prodpool.tile([128, 4 * C], BF16)
                    prod2_v = prod2[:].rearrange("p (n h d) -> p n h d", n=GSZ, h=H)
                    nc.vector.tensor_mul(prod2_v, v_v, p_v)
                    nc.vector.tensor_add(ob_v, ob_v, prod2_v)

            # ---- transpose O to feature-major oT: KT tiles [128c, (n, g)] ----
            ot = otpool.tile([128, KT * WIN], BF16)
            for j in range(KT):
                pst = ps_t.tile([128, WIN], BF16)
                for n in range(GSZ):
                    nc.tensor.transpose(
                        pst[:, n * 128:(n + 1) * 128],
                        ob[:, n * C + j * 128: n * C + (j + 1) * 128],
                        ident_sb[:],
                    )
                nc.scalar.copy(ot[:, j * WIN:(j + 1) * WIN], pst[:])

            # ---- proj matmul + bias, DMA out ----
            for n in range(GSZ):
                for ch in range(2):
                    psy = ps_y.tile([128, 512], F32)
                    for k in range(KT):
                        nc.tensor.matmul(
                            psy[:],
                            lhsT=ot[:, k * WIN + n * 128: k * WIN + (n + 1) * 128],
                            rhs=wproj_sb[:, k * C + ch * 512: k * C + (ch + 1) * 512],
                            start=(k == 0),
                            stop=False,
                        )
                    nc.tensor.matmul(
                        psy[:],
                        lhsT=ones_sb[:1, :],
                        rhs=bias_sb[:1, ch * 512:(ch + 1) * 512],
                        start=False,
                        stop=True,
                    )
                    y_t = ypool.tile([128, 512], F32)
                    nc.vector.tensor_copy(y_t[:], psy[:])
                    nc.sync.dma_start(
                        y[w * WIN + n: w * WIN + WIN: GSZ,
                          ch * 512:(ch + 1) * 512],
                        y_t[:],
                    )


def build_nc(t_core=T_CORE):
    nc = bacc.Bacc("TRN2", target_bir_lowering=False, debug=False)
    x_d = nc.dram_tensor("x", [t_core, C], BF16, kind="ExternalInput")
    wqkv_d = nc.dram_tensor("wqkv", [C, OUT3], BF16, kind="ExternalInput")
    wproj_d = nc.dram_tensor("wproj", [C, C], BF16, kind="ExternalInput")
    bias_d = nc.dram_tensor("bias", [1, C], BF16, kind="ExternalInput")
    ident_d = nc.dram_tensor("ident", [128, 128], BF16, kind="ExternalInput")
    ones_d = nc.dram_tensor("ones", [1, 128], BF16, kind="ExternalInput")
    y_d = nc.dram_tensor("y", [t_core, C], F32, kind="ExternalOutput")
    with tile.TileContext(nc) as tc:
        group_attn_kernel(
            tc, y_d[:], x_d[:], wqkv_d[:], wproj_d[:], bias_d[:],
            ident_d[:], ones_d[:], t_core=t_core,
        )
    nc.compile()
    return nc


def make_in_maps(x, w_qkv, w_proj, b_proj):
    bf = ml_dtypes.bfloat16
    xf = np.ascontiguousarray(np.asarray(x, dtype=np.float32)).reshape(-1, C)
    wq = np.array(w_qkv, dtype=np.float32, copy=True)
    wq[:, :C] *= DH ** -0.5  # fold attention scale into q columns
    wqb = wq.astype(bf)
    wpb = np.asarray(w_proj, dtype=np.float32).astype(bf)
    bb = np.asarray(b_proj, dtype=np.float32).reshape(1, C).astype(bf)
    ident = np.eye(128, dtype=np.float32).astype(bf)
    ones = np.ones((1, 128), dtype=np.float32).astype(bf)
    xb = xf.astype(bf)
    return [
        {
            "x": np.ascontiguousarray(xb[i * T_CORE:(i + 1) * T_CORE]),
            "wqkv": wqb,
            "wproj": wpb,
            "bias": bb,
            "ident": ident,
            "ones": ones,
        }
        for i in range(NCORES)
    ]


_NC_CACHE = {}


def _get_nc():
    if "nc" not in _NC_CACHE:
        _NC_CACHE["nc"] = build_nc()
    return _NC_CACHE["nc"]


def kernel(x, w_qkv, w_proj, b_proj, causal=0, **_unused):
    nc = _get_nc()
    in_maps = make_in_maps(x, w_qkv, w_proj, b_proj)
    res = run_bass_kernel_spmd(nc, in_maps, core_ids=list(range(NCORES)))
    y = np.concatenate([r["y"] for r in res.results], axis=0)
    return y.reshape(B, N, C).astype(np.float32)
